# revision 27
# baseline (speedup 1.0000x reference)
"""BlockSparseRingMultiheadDilatedAttention Trainium2 kernel (v3).

Problem (hardcoded): B=1, N=8192, E=1024, H=16 heads, D=64.
Two dilated groups: g0 = heads 0-7, seg 2048, dilation 1;
                    g1 = heads 8-15, seg 4096, dilation 2, offset 1 (odd positions).
Causal within each (gathered) segment.

Sharding over 8 cores (uniform SPMD program, per-core data):
  core c: a = c%2, sc = c//2, b = c%4, rc = c//4
    g0: seg sc (rows 2048*sc .. +2048), heads 4a..4a+4   (4 blocks of [2048 x 2048])
    g1: seg rc odd rows (gathered, 2048 rows), heads 8+2b..+2 (2 blocks)
  Host pre-slices inputs (bf16 cast, odd-row gather, weight head slices,
  SBUF-layout rearrange) so the device program is identical on every core.
  Host sums the per-core partial output projections (disjoint head
  contributions, bf16) and adds bo.

v3 device dataflow:
  - few LARGE input DMAs (2 per x tensor, 1 per weight tensor) in final SBUF
    layout -- per-DMA fixed costs gated startup in v1/v2 (single HW queue,
    ~2us/DMA completion latency)
  - attention processes BOTH heads of a pair per chunk: two row-tiled
    (tile_position (0,0)/(64,0)) K=64 S-matmuls run concurrently in the PE
    array, land in one 2-bank PSUM tile -> ONE exp per [128,1024]
  - softmax: denominators from the appended-ones row of the PV matmul;
    numerators copied UNNORMALIZED to oT (releases PSUM fast); denominator
    rows gathered onto 8 partitions of one tile -> ONE batched DVE
    reciprocal per pair; broadcast+normalize muls run as fillers inside the
    NEXT pair's window (keeps the DVE FIFO from stalling PE -> HAM warm)
  - pair-2 qkv projections + V transposes are fillers inside pair 0/1's
    ACT(exp)-limited windows; y0/y1 out-proj chunks stream inside pair 2
  - outputs in bf16 (host combines in fp32)
"""

import numpy as np
import ml_dtypes
from collections import deque

BF16 = ml_dtypes.bfloat16

SEG = 2048          # rows per attention block (both groups, post-gather)
E = 1024            # embedding
NQ = 512            # tq chunk (one PSUM bank of fp32)
NTQ = SEG // NQ     # 4 tq chunks per block
NTK = SEG // 128    # 16 tk chunks per block
ECH = E // 128      # 8 embedding chunks

_CACHE = {}


def _build_program():
    import concourse.bacc as bacc
    import concourse.mybir as mybir
    import concourse.tile as tile

    dt = mybir.dt
    nc = bacc.Bacc("TRN2", target_bir_lowering=False, debug=False,
                   enable_asserts=False)

    # ---- DRAM I/O (uniform across cores; host slices per core) ----
    # x tensors already in SBUF layout [128, 2(pos half) * 8(ec) * 1024(pos)]
    xs = {}
    for sel in ("a", "b"):
        for inp in ("q", "k", "v"):
            xs[(sel, inp)] = nc.dram_tensor(
                f"x{sel}_{inp}", [128, 2 * ECH * 1024], dt.bfloat16,
                kind="ExternalInput").ap()
    ws = {inp: nc.dram_tensor(f"w{inp}", [128, 384 * ECH], dt.bfloat16,
                              kind="ExternalInput").ap()
          for inp in ("q", "k", "v")}
    wo = nc.dram_tensor("wo", [128, 3 * E], dt.bfloat16,
                        kind="ExternalInput").ap()
    ball = nc.dram_tensor("ball", [128, 9], dt.float32,
                          kind="ExternalInput").ap()
    y0 = nc.dram_tensor("y0", [SEG, E], dt.bfloat16, kind="ExternalOutput").ap()
    y1 = nc.dram_tensor("y1", [SEG, E], dt.bfloat16, kind="ExternalOutput").ap()

    with tile.TileContext(nc) as tc:
        from contextlib import ExitStack
        with ExitStack() as ctx:
            const = ctx.enter_context(tc.tile_pool(name="const", bufs=1))
            wpool = ctx.enter_context(tc.tile_pool(name="wpool", bufs=1))
            xpool = ctx.enter_context(tc.tile_pool(name="xpool", bufs=4))
            vtp = ctx.enter_context(tc.tile_pool(name="vtp", bufs=2))
            qkt = ctx.enter_context(tc.tile_pool(name="qkt", bufs=1))
            vnat = ctx.enter_context(tc.tile_pool(name="vnat", bufs=1))
            otp = ctx.enter_context(tc.tile_pool(name="otp", bufs=1))
            ptp = ctx.enter_context(tc.tile_pool(name="ptp", bufs=3))
            dnp = ctx.enter_context(tc.tile_pool(name="dnp", bufs=1))
            dsp_p = ctx.enter_context(tc.tile_pool(name="dsp_p", bufs=1))
            rcq = ctx.enter_context(tc.tile_pool(name="rcq", bufs=2))
            rbp = ctx.enter_context(tc.tile_pool(name="rbp", bufs=2))
            rbtp = ctx.enter_context(tc.tile_pool(name="rbtp", bufs=1))
            ysb = ctx.enter_context(tc.tile_pool(name="ysb", bufs=2))
            ps_s = ctx.enter_context(
                tc.tile_pool(name="ps_s", bufs=2, space="PSUM"))
            ps_b = ctx.enter_context(
                tc.tile_pool(name="ps_b", bufs=1, space="PSUM"))
            ps_o = ctx.enter_context(
                tc.tile_pool(name="ps_o", bufs=2, space="PSUM"))

            # ---- constants: identity (PE transpose), causal chunk masks ----
            ident = const.tile([128, 128], dt.bfloat16, tag="ident")
            nc.gpsimd.memset(ident, 1.0)
            nc.gpsimd.affine_select(
                out=ident, in_=ident, compare_op=mybir.AluOpType.is_equal,
                fill=0.0, base=0, pattern=[[-1, 128]], channel_multiplier=1)
            # additive causal mask for the 128-wide diagonal strip of a
            # diagonal k-chunk (both heads): 0 where f - p >= 0, else -1e9.
            # Applied to S in PSUM right after the S matmuls (concurrent
            # with the previous chunk's exp -> off the critical chain).
            maskadd = const.tile([128, 256], dt.bfloat16, tag="maskadd")
            nc.gpsimd.memset(maskadd, 0.0)
            nc.gpsimd.affine_select(
                out=maskadd.rearrange("p (hh f) -> p hh f", hh=2),
                in_=maskadd.rearrange("p (hh f) -> p hh f", hh=2),
                compare_op=mybir.AluOpType.is_ge,
                fill=-1e9, base=0, pattern=[[0, 2], [1, 128]],
                channel_multiplier=-1)

            # ---- PE warmup: keep HAM at full clock through the
            # DMA-gated startup (these run while inputs stream in) ----
            dum = const.tile([128, NQ], dt.bfloat16, tag="dum")
            nc.gpsimd.memset(dum, 0.0)
            wps = ps_b.tile([128, 2 * NQ], dt.float32, tag="b",
                            name="warm")
            for _ in range(75):
                nc.tensor.matmul(wps[:, 0:NQ], ident, dum,
                                 start=True, stop=True)

            # ---- weights: one DMA per tensor (host pre-layouts) ----
            wTa = {}
            for inp in ("q", "k", "v"):
                t = wpool.tile([128, 384 * ECH], dt.bfloat16,
                               tag=f"wT_{inp}", name=f"wT_{inp}")
                wTa[inp] = t
            wT = {inp: [wTa[inp].rearrange("p (ec x) -> p ec x", x=384)
                        [:, :, 128 * p:128 * (p + 1)]
                        for p in range(3)] for inp in ("q", "k", "v")}
            woTa = wpool.tile([128, 3 * E], dt.bfloat16, tag="woT")
            nc.sync.dma_start(out=woTa, in_=wo)
            woT = [woTa[:, E * p:E * (p + 1)] for p in range(3)]
            ballt = wpool.tile([128, 9], dt.float32, tag="ball")
            nc.sync.dma_start(out=ballt, in_=ball)
            bsb = {}
            for i, inp in enumerate(("q", "k", "v")):
                for p in range(3):
                    bsb[(inp, p)] = ballt[:, 3 * i + p:3 * i + p + 1]

            # ---- persistent per-pair activations ----
            qT = [qkt.tile([128, SEG], dt.bfloat16, tag=f"qT{p}", name=f"qT{p}")
                  for p in range(3)]
            kT = [qkt.tile([128, SEG], dt.bfloat16, tag=f"kT{p}", name=f"kT{p}")
                  for p in range(3)]
            vn = [vnat.tile([128, NTK * 130], dt.bfloat16, tag=f"vn{p}",
                            name=f"vn{p}")
                  for p in range(3)]
            for p in range(3):
                ones_view = vn[p].rearrange("p (k x) -> p k x", x=130)
                nc.gpsimd.memset(ones_view[:, :, 64:65], 1.0)
                nc.gpsimd.memset(ones_view[:, :, 129:130], 1.0)
            oT = [otp.tile([128, SEG], dt.bfloat16, tag=f"oT{p}", name=f"oT{p}")
                  for p in range(3)]

            # ---- input loads: 2 DMAs (position halves) per tensor ----
            xt_tiles = {}
            HC = ECH * 1024     # columns per position half

            def load_x_half(sel, inp, t2):
                t = xpool.tile([128, HC], dt.bfloat16, tag="xt", name="xt")
                nc.sync.dma_start(
                    out=t, in_=xs[(sel, inp)][:, HC * t2:HC * (t2 + 1)])
                xt_tiles[(sel, inp, t2)] = t.rearrange(
                    "p (ec s) -> p ec s", ec=ECH)
            # xa halves interleaved (t2=0 of q,k,v first, each preceded
            # by its weight tensor) so attention can start after the first
            # position-half is projected; xb after
            for inp in ("q", "k", "v"):
                load_x_half("a", inp, 0)
                nc.sync.dma_start(out=wTa[inp], in_=ws[inp])
            for inp in ("q", "k", "v"):
                load_x_half("a", inp, 1)
            for inp in ("q", "k", "v"):
                for t2 in range(2):
                    load_x_half("b", inp, t2)

            # ---- projection building blocks ----
            def proj_half(acc, inp, p, t2, half, sel):
                xt = xt_tiles[(sel, inp, t2)]
                for ec in range(ECH):
                    nc.tensor.matmul(
                        acc[:, NQ * half:NQ * (half + 1)],
                        wT[inp][p][:, ec, :],
                        xt[:, ec, NQ * half:NQ * (half + 1)],
                        start=(ec == 0), stop=(ec == ECH - 1))

            def vtrans_batch(p, i0, vtile):
                # transpose 4 [128,128] chunks of vT into V-natural slices
                ptr = ps_b.tile([128, 4 * 128], dt.bfloat16, tag="b",
                                name="ptr")
                for qq in range(4):
                    nc.tensor.transpose(
                        ptr[:, 128 * qq:128 * (qq + 1)],
                        vtile[:, 128 * (i0 + qq):128 * (i0 + qq + 1)], ident)
                src = ptr.rearrange("p (c h d) -> p c h d", c=4, h=2)
                dst = vn[p][:, 130 * i0:130 * (i0 + 4)].rearrange(
                    "p (c h x) -> p c h x", c=4, x=65)[:, :, :, 0:64]
                nc.vector.tensor_copy(dst, src)

            # ---- proj group: one [128,1024] output col-block ----
            def proj_group(inp, p, t2, dst, sel, act_bias):
                acc = (ps_s if act_bias else ps_b).tile(
                    [128, 2 * NQ], dt.float32, tag=("s" if act_bias else "b"),
                    name="proj")
                proj_half(acc, inp, p, t2, 0, sel)
                proj_half(acc, inp, p, t2, 1, sel)
                if act_bias:
                    nc.scalar.activation(
                        dst[:, 2 * NQ * t2:2 * NQ * (t2 + 1)], acc,
                        mybir.ActivationFunctionType.Identity,
                        bias=bsb[(inp, p)], scale=1.0)
                else:
                    nc.vector.tensor_scalar_add(
                        dst[:, 2 * NQ * t2:2 * NQ * (t2 + 1)], acc,
                        bsb[(inp, p)])

            # ---- Phase A: t2=0 projections for pairs 0,1 (pre-attention) --
            vts = {}
            for inp in ("q", "k", "v"):
                for p in (0, 1):
                    if inp == "v":
                        dst = vtp.tile([128, SEG], dt.bfloat16, tag="vT",
                                       name="vT")
                        vts[p] = dst
                    else:
                        dst = (qT if inp == "q" else kT)[p]
                    proj_group(inp, p, 0, dst, "a", True)
                    if inp == "v":
                        vtrans_batch(p, 0, dst)
                        vtrans_batch(p, 4, dst)

            # ---- filler machinery (global slot counter, absolute gates) --
            fillers = []        # (min_slot, thunk)
            slot_counter = [0]

            def slot_cb():
                slot_counter[0] += 1
                s = slot_counter[0]
                for i in range(len(fillers)):
                    if fillers[i][0] <= s:
                        th = fillers[i][1]
                        del fillers[i]
                        th()
                        break

            def drain_fillers():
                while fillers:
                    fillers.pop(0)[1]()

            # t2=1 projections for pairs 0,1 -> early attn(0) fillers
            for gi, (inp, p) in enumerate(
                    (("q", 0), ("q", 1), ("k", 0), ("k", 1),
                     ("v", 0), ("v", 1))):
                dst = vts[p] if inp == "v" else (qT if inp == "q" else kT)[p]
                fillers.append(
                    (2 + gi, lambda inp=inp, p=p, dst=dst:
                     proj_group(inp, p, 1, dst, "a", True)))
            for gi, (p, i0) in enumerate(((0, 8), (0, 12), (1, 8), (1, 12))):
                fillers.append(
                    (9 + gi, lambda p=p, i0=i0: vtrans_batch(p, i0, vts[p])))

            # pair-2 projections (xb) as later attn(0) fillers
            def make_projB_fillers():
                vtile = vtp.tile([128, SEG], dt.bfloat16, tag="vT",
                                 name="vtB")
                gates = {"q": 14, "k": 22, "v": 41}
                for inp in ("q", "k", "v"):
                    dst = {"q": qT[2], "k": kT[2], "v": vtile}[inp]
                    g = gates[inp]
                    for t2 in range(2):
                        fillers.append(
                            (g + 3 * t2, lambda inp=inp, t2=t2, dst=dst:
                             proj_group(inp, 2, t2, dst, "b", False)))
                for bi, i0 in enumerate((0, 4, 8, 12)):
                    fillers.append(
                        (47 + 2 * bi, lambda i0=i0: vtrans_batch(2, i0,
                                                                 vtile)))

            # out-projection chunk fillers
            def y_unit(ydram, pairs, m, cast_on_act=False):
                def th():
                    accy = ps_b.tile([128, 2 * NQ], dt.float32, tag="b",
                                     name="accy")
                    for jc in range(2):
                        for idx, p in enumerate(pairs):
                            nc.tensor.matmul(
                                accy[:, NQ * jc:NQ * (jc + 1)],
                                oT[p][:, 128 * m:128 * (m + 1)],
                                woT[p][:, NQ * jc:NQ * (jc + 1)],
                                start=(idx == 0), stop=(idx == len(pairs) - 1))
                    t = ysb.tile([128, 2 * NQ], dt.bfloat16, tag="ysb")
                    if cast_on_act:
                        nc.scalar.copy(t, accy)
                    else:
                        nc.vector.tensor_copy(t, accy)
                    nc.sync.dma_start(
                        out=ydram[128 * m:128 * (m + 1), :], in_=t)
                return th

            # ---- normalization (DMA-scatter batched reciprocal) ----
            def drow(j, h):
                return 32 * (2 * (j % 2) + h), NQ * (j // 2)

            dens = [dnp.tile([128, 2 * NQ], dt.bfloat16, tag="dens",
                             name=f"dens{p}") for p in range(3)]
            dsp = [dsp_p.tile([32, 8 * 16], dt.bfloat16, tag="dsp",
                              name=f"dsp{p}") for p in range(3)]
            rsp = [dsp_p.tile([32, 8 * 16], dt.float32, tag="rsp",
                              name=f"rsp{p}") for p in range(3)]

            def recip_batch(p, us):
                # spread each den row over 32 partitions (64B DMA lines) so
                # ONE short-free-dim reciprocal covers the batch
                def th():
                    for u in us:
                        r, c = drow(u // 2, u % 2)
                        nc.gpsimd.dma_start(
                            out=dsp[p][:, 16 * u:16 * (u + 1)],
                            in_=dens[p][r:r + 1, c:c + NQ])
                    u0, u1 = us[0], us[-1] + 1
                    nc.vector.reciprocal(rsp[p][:, 16 * u0:16 * u1],
                                         dsp[p][:, 16 * u0:16 * u1])
                return th

            def norm_bc_unit(p, j, rb2s):
                def th():
                    rb2 = rbp.tile([128, NQ], dt.float32, tag="rb")
                    rb2s[j] = rb2
                    for h in range(2):
                        u = 2 * j + h
                        rc = rcq.tile([1, NQ], dt.float32, tag="rc")
                        nc.gpsimd.dma_start(
                            out=rc, in_=rsp[p][:, 16 * u:16 * (u + 1)])
                        if h == 0:
                            nc.gpsimd.partition_broadcast(rb2[0:64, :], rc)
                        else:
                            rbt = rbtp.tile([64, NQ], dt.float32,
                                            tag="rbt")
                            nc.gpsimd.partition_broadcast(rbt, rc)
                            nc.gpsimd.dma_start(out=rb2[64:128, :], in_=rbt)
                return th

            def norm_mul_unit(p, j, rb2s):
                def th():
                    sl = oT[p][:, NQ * j:NQ * (j + 1)]
                    nc.vector.tensor_mul(sl, sl, rb2s.pop(j))
                return th

            def norm_fillers(p, base, js):
                rb2s = {}
                ents = [(base, recip_batch(p, [u for j in js
                                               for u in (2 * j, 2 * j + 1)]))]
                for i, j in enumerate(js):
                    ents.append((base + 2 + i, norm_bc_unit(p, j, rb2s)))
                    ents.append((base + 4 + i, norm_mul_unit(p, j, rb2s)))
                ents.sort(key=lambda e: e[0])
                return ents

            # ---- attention: flat chunk stream with cross-boundary S
            # lookahead (keeps ACT fed across j/pair boundaries) ----
            def s_emit(p, j, ci):
                s = ps_s.tile([128, 2 * NQ], dt.float32, tag="s", name="s")
                c0 = max(ci - 4 * j, 0) * 128
                nc.tensor.matmul(
                    s[:, c0:NQ],
                    kT[p][0:64, 128 * ci:128 * (ci + 1)],
                    qT[p][0:64, NQ * j + c0:NQ * (j + 1)],
                    start=True, stop=True)
                nc.tensor.matmul(
                    s[:, NQ + c0:2 * NQ],
                    kT[p][64:128, 128 * ci:128 * (ci + 1)],
                    qT[p][64:128, NQ * j + c0:NQ * (j + 1)],
                    start=True, stop=True)
                di = ci - 4 * j
                if di >= 0:
                    sv = s.rearrange("p (h x) -> p h x", h=2)[
                        :, :, c0:c0 + 128]
                    nc.vector.tensor_add(
                        sv, sv,
                        maskadd.rearrange("p (h x) -> p h x", h=2))
                return s

            chunks = []         # flat stream: (p, j, ci, nchunks)
            hooks = {}          # (p, j) -> after_j hook
            for p in range(3):
                js = range(NTQ) if p < 2 else range(NTQ - 1, -1, -1)
                for j in js:
                    n = 4 * (j + 1)
                    for ci in range(n):
                        chunks.append((p, j, ci, n))

            def attn_all():
                accs = {}
                s_tiles = {}
                first = chunks[0]
                s_tiles[first[:3]] = s_emit(*first[:3])
                for idx, (p, j, ci, n) in enumerate(chunks):
                    if ci == 0:
                        acc0 = ps_o.tile([128, NQ], dt.float32, tag="acc",
                                         name="acc0")
                        acc1 = ps_o.tile([128, NQ], dt.float32, tag="acc",
                                         name="acc1")
                        accs[(p, j)] = (acc0, acc1)
                    acc0, acc1 = accs[(p, j)]
                    s = s_tiles.pop((p, j, ci))
                    pt = ptp.tile([128, 2 * NQ], dt.bfloat16, tag="pt",
                                  name="pt")
                    # cols [0, 128*di) of a diagonal chunk are fully masked:
                    # exp and the PV matmuls skip them entirely
                    c0 = max(ci - 4 * j, 0) * 128
                    if c0 == 0:
                        nc.scalar.activation(
                            pt, s, mybir.ActivationFunctionType.Exp,
                            bias=0.0, scale=0.125)
                    else:
                        pv = pt.rearrange("p (h x) -> p h x", h=2)[
                            :, :, c0:NQ]
                        sv = s.rearrange("p (h x) -> p h x", h=2)[
                            :, :, c0:NQ]
                        nc.scalar.activation(
                            pv, sv, mybir.ActivationFunctionType.Exp,
                            bias=0.0, scale=0.125)
                    if idx + 1 < len(chunks):
                        nxt = chunks[idx + 1]
                        s_tiles[nxt[:3]] = s_emit(*nxt[:3])
                    last = (ci == n - 1)
                    nc.tensor.matmul(
                        acc0[0:65, c0:NQ],
                        vn[p][:, 130 * ci:130 * ci + 65],
                        pt[:, c0:NQ], start=(ci == 0), stop=last)
                    nc.tensor.matmul(
                        acc1[0:65, c0:NQ],
                        vn[p][:, 130 * ci + 65:130 * ci + 130],
                        pt[:, NQ + c0:2 * NQ], start=(ci == 0), stop=last)
                    slot_cb()
                    if last:
                        for h, acc in enumerate((acc0, acc1)):
                            nc.vector.tensor_copy(
                                oT[p][64 * h:64 * h + 64,
                                      NQ * j:NQ * (j + 1)],
                                acc[0:64, :])
                            r, c = drow(j, h)
                            nc.vector.tensor_copy(
                                dens[p][r:r + 1, c:c + NQ], acc[64:65, :])
                        accs.pop((p, j))
                        hk = hooks.get((p, j))
                        if hk is not None:
                            hk()

            # norm/y filler schedule: pairs 0/1 normalize per j-half;
            # pair 2 runs j descending and normalizes each j right after
            # it completes, so y1 streams inside the pair (small tail)
            def q_norm(p, js, yd=None, pairs=None, ms=()):
                def hk():
                    base = slot_counter[0] + 1
                    fillers.extend(norm_fillers(p, base, js))
                    for i, m in enumerate(ms):
                        fillers.append((base + 6 + i, y_unit(yd, pairs, m)))
                return hk

            hooks[(0, 1)] = q_norm(0, [0, 1])
            hooks[(0, 3)] = q_norm(0, [2, 3])
            hooks[(1, 1)] = q_norm(1, [0, 1], y0, (0, 1), range(8))
            hooks[(1, 3)] = q_norm(1, [2, 3], y0, (0, 1), range(8, 16))
            hooks[(2, 3)] = q_norm(2, [3], y1, (2,), range(12, 16))
            hooks[(2, 2)] = q_norm(2, [2], y1, (2,), range(8, 12))
            hooks[(2, 1)] = q_norm(2, [1], y1, (2,), range(4, 8))
            hooks[(2, 0)] = q_norm(2, [0], y1, (2,), range(0, 4))

            make_projB_fillers()
            attn_all()
            drain_fillers()

    nc.compile()
    return nc


def _get_program():
    if "nc" not in _CACHE:
        _CACHE["nc"] = _build_program()
    return _CACHE["nc"]


def _sbuf_layout(xT):
    """[1024, 2048] (e, pos) -> [128, 2*8*1024]: pos-half major, ec, pos."""
    # [ec, 128, t2, 1024] -> [128, t2, ec, 1024]
    return np.ascontiguousarray(
        xT.reshape(ECH, 128, 2, 1024).transpose(1, 2, 0, 3).reshape(
            128, 2 * ECH * 1024))


def _prep_inputs(query, key, value, Wq, bq, Wk, bk, Wv, bv, Wo, bo):
    """Build the 8 per-core input maps (host-side slicing + bf16 cast)."""
    q = np.asarray(query, np.float32).reshape(8192, 1024).astype(BF16)
    k = np.asarray(key, np.float32).reshape(8192, 1024).astype(BF16)
    v = np.asarray(value, np.float32).reshape(8192, 1024).astype(BF16)
    wq = np.asarray(Wq, np.float32).astype(BF16)
    wk = np.asarray(Wk, np.float32).astype(BF16)
    wv = np.asarray(Wv, np.float32).astype(BF16)
    wo_f = np.asarray(Wo, np.float32).astype(BF16)
    bqf = np.asarray(bq, np.float32)
    bkf = np.asarray(bk, np.float32)
    bvf = np.asarray(bv, np.float32)

    qT, kT, vT = q.T, k.T, v.T  # [1024, 8192] views
    in_maps = []
    for c in range(8):
        a, sc, b, rc = c % 2, c // 2, c % 4, c // 4
        rows_g0 = slice(2048 * sc, 2048 * (sc + 1))
        rows_g1 = slice(4096 * rc + 1, 4096 * (rc + 1), 2)
        hrows = np.r_[256 * a:256 * a + 256, 512 + 128 * b:512 + 128 * b + 128]

        def wlay(w):
            # [1024, 384] -> [128, 8*384] (ec-blocked)
            return np.ascontiguousarray(
                w.reshape(ECH, 128, 384).transpose(1, 0, 2).reshape(128, -1))

        wov = np.ascontiguousarray(wo_f[:, hrows].T)  # [384, 1024]
        ballv = np.stack([np.ascontiguousarray(bf[hrows]).reshape(3, 128)
                          for bf in (bqf, bkf, bvf)], axis=0)  # [3,3,128]
        # ball[128, 9]: col 3i+p = input i, pair p
        ballv = np.ascontiguousarray(ballv.reshape(9, 128).T)
        m = {
            "xa_q": _sbuf_layout(np.ascontiguousarray(qT[:, rows_g0])),
            "xa_k": _sbuf_layout(np.ascontiguousarray(kT[:, rows_g0])),
            "xa_v": _sbuf_layout(np.ascontiguousarray(vT[:, rows_g0])),
            "xb_q": _sbuf_layout(np.ascontiguousarray(qT[:, rows_g1])),
            "xb_k": _sbuf_layout(np.ascontiguousarray(kT[:, rows_g1])),
            "xb_v": _sbuf_layout(np.ascontiguousarray(vT[:, rows_g1])),
            "wq": wlay(np.ascontiguousarray(wq[hrows].T)),
            "wk": wlay(np.ascontiguousarray(wk[hrows].T)),
            "wv": wlay(np.ascontiguousarray(wv[hrows].T)),
            "wo": np.ascontiguousarray(
                wov.reshape(3, 128, 1024).transpose(1, 0, 2).reshape(128, -1)),
            "ball": ballv,
        }
        in_maps.append(m)
    return in_maps


def _combine(results, bo):
    y = np.zeros((8192, 1024), np.float32)
    for c in range(8):
        sc, rc = c // 2, c // 4
        y[2048 * sc:2048 * (sc + 1)] += results[c]["y0"].astype(np.float32)
        y[4096 * rc + 1:4096 * (rc + 1):2] += results[c]["y1"].astype(np.float32)
    y += np.asarray(bo, np.float32)
    return y.reshape(1, 8192, 1024)


def kernel(query, key, value, Wq, bq, Wk, bk, Wv, bv, Wo, bo,
           _trace=False, _trace_cores=None):
    from concourse import bass_utils
    nc = _get_program()
    in_maps = _prep_inputs(query, key, value, Wq, bq, Wk, bk, Wv, bv, Wo, bo)
    res = bass_utils.run_bass_kernel_spmd(
        nc, in_maps, core_ids=list(range(8)),
        trace=_trace, trace_cores=_trace_cores)
    _CACHE["last_results"] = res
    return _combine(res.results, bo)


# revision 28
# speedup vs baseline: 1.0109x; 1.0109x over previous
"""BlockSparseRingMultiheadDilatedAttention Trainium2 kernel (v3).

Problem (hardcoded): B=1, N=8192, E=1024, H=16 heads, D=64.
Two dilated groups: g0 = heads 0-7, seg 2048, dilation 1;
                    g1 = heads 8-15, seg 4096, dilation 2, offset 1 (odd positions).
Causal within each (gathered) segment.

Sharding over 8 cores (uniform SPMD program, per-core data):
  core c: a = c%2, sc = c//2, b = c%4, rc = c//4
    g0: seg sc (rows 2048*sc .. +2048), heads 4a..4a+4   (4 blocks of [2048 x 2048])
    g1: seg rc odd rows (gathered, 2048 rows), heads 8+2b..+2 (2 blocks)
  Host pre-slices inputs (bf16 cast, odd-row gather, weight head slices,
  SBUF-layout rearrange) so the device program is identical on every core.
  Host sums the per-core partial output projections (disjoint head
  contributions, bf16) and adds bo.

v3 device dataflow:
  - few LARGE input DMAs (2 per x tensor, 1 per weight tensor) in final SBUF
    layout -- per-DMA fixed costs gated startup in v1/v2 (single HW queue,
    ~2us/DMA completion latency)
  - attention processes BOTH heads of a pair per chunk: two row-tiled
    (tile_position (0,0)/(64,0)) K=64 S-matmuls run concurrently in the PE
    array, land in one 2-bank PSUM tile -> ONE exp per [128,1024]
  - softmax: denominators from the appended-ones row of the PV matmul;
    numerators copied UNNORMALIZED to oT (releases PSUM fast); denominator
    rows gathered onto 8 partitions of one tile -> ONE batched DVE
    reciprocal per pair; broadcast+normalize muls run as fillers inside the
    NEXT pair's window (keeps the DVE FIFO from stalling PE -> HAM warm)
  - pair-2 qkv projections + V transposes are fillers inside pair 0/1's
    ACT(exp)-limited windows; y0/y1 out-proj chunks stream inside pair 2
  - outputs in bf16 (host combines in fp32)
"""

import numpy as np
import ml_dtypes
from collections import deque

BF16 = ml_dtypes.bfloat16

SEG = 2048          # rows per attention block (both groups, post-gather)
E = 1024            # embedding
NQ = 512            # tq chunk (one PSUM bank of fp32)
NTQ = SEG // NQ     # 4 tq chunks per block
NTK = SEG // 128    # 16 tk chunks per block
ECH = E // 128      # 8 embedding chunks

_CACHE = {}


def _build_program():
    import concourse.bacc as bacc
    import concourse.mybir as mybir
    import concourse.tile as tile

    dt = mybir.dt
    nc = bacc.Bacc("TRN2", target_bir_lowering=False, debug=False,
                   enable_asserts=False)

    # ---- DRAM I/O (uniform across cores; host slices per core) ----
    # x tensors already in SBUF layout [128, 2(pos half) * 8(ec) * 1024(pos)]
    xs = {}
    for sel in ("a", "b"):
        for inp in ("q", "k", "v"):
            xs[(sel, inp)] = nc.dram_tensor(
                f"x{sel}_{inp}", [128, 2 * ECH * 1024], dt.bfloat16,
                kind="ExternalInput").ap()
    ws = {inp: nc.dram_tensor(f"w{inp}", [128, 384 * ECH], dt.bfloat16,
                              kind="ExternalInput").ap()
          for inp in ("q", "k", "v")}
    wo = nc.dram_tensor("wo", [128, 3 * E], dt.bfloat16,
                        kind="ExternalInput").ap()
    ball = nc.dram_tensor("ball", [128, 9], dt.float32,
                          kind="ExternalInput").ap()
    y0 = nc.dram_tensor("y0", [SEG, E], dt.bfloat16, kind="ExternalOutput").ap()
    y1 = nc.dram_tensor("y1", [SEG, E], dt.bfloat16, kind="ExternalOutput").ap()

    with tile.TileContext(nc) as tc:
        from contextlib import ExitStack
        with ExitStack() as ctx:
            const = ctx.enter_context(tc.tile_pool(name="const", bufs=1))
            wpool = ctx.enter_context(tc.tile_pool(name="wpool", bufs=1))
            xpool = ctx.enter_context(tc.tile_pool(name="xpool", bufs=4))
            vtp = ctx.enter_context(tc.tile_pool(name="vtp", bufs=2))
            qkt = ctx.enter_context(tc.tile_pool(name="qkt", bufs=1))
            vnat = ctx.enter_context(tc.tile_pool(name="vnat", bufs=1))
            otp = ctx.enter_context(tc.tile_pool(name="otp", bufs=1))
            ptp = ctx.enter_context(tc.tile_pool(name="ptp", bufs=3))
            dnp = ctx.enter_context(tc.tile_pool(name="dnp", bufs=1))
            dsp_p = ctx.enter_context(tc.tile_pool(name="dsp_p", bufs=1))
            rcq = ctx.enter_context(tc.tile_pool(name="rcq", bufs=2))
            rbp = ctx.enter_context(tc.tile_pool(name="rbp", bufs=2))
            rbtp = ctx.enter_context(tc.tile_pool(name="rbtp", bufs=1))
            ysb = ctx.enter_context(tc.tile_pool(name="ysb", bufs=2))
            ps_s = ctx.enter_context(
                tc.tile_pool(name="ps_s", bufs=2, space="PSUM"))
            ps_b = ctx.enter_context(
                tc.tile_pool(name="ps_b", bufs=1, space="PSUM"))
            ps_o = ctx.enter_context(
                tc.tile_pool(name="ps_o", bufs=2, space="PSUM"))

            # ---- constants: identity (PE transpose), causal chunk masks ----
            ident = const.tile([128, 128], dt.bfloat16, tag="ident")
            nc.gpsimd.memset(ident, 1.0)
            nc.gpsimd.affine_select(
                out=ident, in_=ident, compare_op=mybir.AluOpType.is_equal,
                fill=0.0, base=0, pattern=[[-1, 128]], channel_multiplier=1)
            # additive causal mask for the 128-wide diagonal strip of a
            # diagonal k-chunk (both heads): 0 where f - p >= 0, else -1e9.
            # Applied to S in PSUM right after the S matmuls (concurrent
            # with the previous chunk's exp -> off the critical chain).
            maskadd = const.tile([128, 256], dt.bfloat16, tag="maskadd")
            nc.gpsimd.memset(maskadd, 0.0)
            nc.gpsimd.affine_select(
                out=maskadd.rearrange("p (hh f) -> p hh f", hh=2),
                in_=maskadd.rearrange("p (hh f) -> p hh f", hh=2),
                compare_op=mybir.AluOpType.is_ge,
                fill=-1e9, base=0, pattern=[[0, 2], [1, 128]],
                channel_multiplier=-1)

            # ---- PE warmup: keep HAM at full clock through the
            # DMA-gated startup (these run while inputs stream in) ----
            dum = const.tile([128, NQ], dt.bfloat16, tag="dum")
            nc.gpsimd.memset(dum, 0.0)
            wps = ps_b.tile([128, 2 * NQ], dt.float32, tag="b",
                            name="warm")
            for _ in range(60):
                nc.tensor.matmul(wps[:, 0:NQ], ident, dum,
                                 start=True, stop=True)

            # ---- weights: one DMA per tensor (host pre-layouts) ----
            wTa = {}
            for inp in ("q", "k", "v"):
                t = wpool.tile([128, 384 * ECH], dt.bfloat16,
                               tag=f"wT_{inp}", name=f"wT_{inp}")
                wTa[inp] = t
            wT = {inp: [wTa[inp].rearrange("p (ec x) -> p ec x", x=384)
                        [:, :, 128 * p:128 * (p + 1)]
                        for p in range(3)] for inp in ("q", "k", "v")}
            woTa = wpool.tile([128, 3 * E], dt.bfloat16, tag="woT")
            nc.sync.dma_start(out=woTa, in_=wo)
            woT = [woTa[:, E * p:E * (p + 1)] for p in range(3)]
            ballt = wpool.tile([128, 9], dt.float32, tag="ball")
            nc.sync.dma_start(out=ballt, in_=ball)
            bsb = {}
            for i, inp in enumerate(("q", "k", "v")):
                for p in range(3):
                    bsb[(inp, p)] = ballt[:, 3 * i + p:3 * i + p + 1]

            # ---- persistent per-pair activations ----
            qT = [qkt.tile([128, SEG], dt.bfloat16, tag=f"qT{p}", name=f"qT{p}")
                  for p in range(3)]
            kT = [qkt.tile([128, SEG], dt.bfloat16, tag=f"kT{p}", name=f"kT{p}")
                  for p in range(3)]
            vn = [vnat.tile([128, NTK * 130], dt.bfloat16, tag=f"vn{p}",
                            name=f"vn{p}")
                  for p in range(3)]
            for p in range(3):
                ones_view = vn[p].rearrange("p (k x) -> p k x", x=130)
                nc.gpsimd.memset(ones_view[:, :, 64:65], 1.0)
                nc.gpsimd.memset(ones_view[:, :, 129:130], 1.0)
            oT = [otp.tile([128, SEG], dt.bfloat16, tag=f"oT{p}", name=f"oT{p}")
                  for p in range(3)]

            # ---- input loads: 2 DMAs (position halves) per tensor ----
            xt_tiles = {}
            HC = ECH * 1024     # columns per position half

            def load_x_half(sel, inp, t2):
                t = xpool.tile([128, HC], dt.bfloat16, tag="xt", name="xt")
                nc.sync.dma_start(
                    out=t, in_=xs[(sel, inp)][:, HC * t2:HC * (t2 + 1)])
                xt_tiles[(sel, inp, t2)] = t.rearrange(
                    "p (ec s) -> p ec s", ec=ECH)
            # xa halves interleaved (t2=0 of q,k,v first, each preceded
            # by its weight tensor) so attention can start after the first
            # position-half is projected; xb after
            for inp in ("q", "k", "v"):
                load_x_half("a", inp, 0)
                nc.sync.dma_start(out=wTa[inp], in_=ws[inp])
            for inp in ("q", "k", "v"):
                load_x_half("a", inp, 1)
            for inp in ("q", "k", "v"):
                for t2 in range(2):
                    load_x_half("b", inp, t2)

            # ---- projection building blocks ----
            def proj_half(acc, inp, p, t2, half, sel):
                xt = xt_tiles[(sel, inp, t2)]
                for ec in range(ECH):
                    nc.tensor.matmul(
                        acc[:, NQ * half:NQ * (half + 1)],
                        wT[inp][p][:, ec, :],
                        xt[:, ec, NQ * half:NQ * (half + 1)],
                        start=(ec == 0), stop=(ec == ECH - 1))

            def vtrans_batch(p, i0, vtile):
                # transpose 4 [128,128] chunks of vT into V-natural slices
                ptr = ps_b.tile([128, 4 * 128], dt.bfloat16, tag="b",
                                name="ptr")
                for qq in range(4):
                    nc.tensor.transpose(
                        ptr[:, 128 * qq:128 * (qq + 1)],
                        vtile[:, 128 * (i0 + qq):128 * (i0 + qq + 1)], ident)
                src = ptr.rearrange("p (c h d) -> p c h d", c=4, h=2)
                dst = vn[p][:, 130 * i0:130 * (i0 + 4)].rearrange(
                    "p (c h x) -> p c h x", c=4, x=65)[:, :, :, 0:64]
                nc.vector.tensor_copy(dst, src)

            # ---- proj group: one [128,1024] output col-block ----
            def proj_group(inp, p, t2, dst, sel, act_bias):
                acc = (ps_s if act_bias else ps_b).tile(
                    [128, 2 * NQ], dt.float32, tag=("s" if act_bias else "b"),
                    name="proj")
                proj_half(acc, inp, p, t2, 0, sel)
                proj_half(acc, inp, p, t2, 1, sel)
                if act_bias:
                    nc.scalar.activation(
                        dst[:, 2 * NQ * t2:2 * NQ * (t2 + 1)], acc,
                        mybir.ActivationFunctionType.Identity,
                        bias=bsb[(inp, p)], scale=1.0)
                else:
                    nc.vector.tensor_scalar_add(
                        dst[:, 2 * NQ * t2:2 * NQ * (t2 + 1)], acc,
                        bsb[(inp, p)])

            # ---- Phase A: t2=0 projections for pairs 0,1 (pre-attention) --
            vts = {}
            for inp in ("q", "k", "v"):
                for p in (0, 1):
                    if inp == "v":
                        dst = vtp.tile([128, SEG], dt.bfloat16, tag="vT",
                                       name="vT")
                        vts[p] = dst
                    else:
                        dst = (qT if inp == "q" else kT)[p]
                    proj_group(inp, p, 0, dst, "a", True)
                    if inp == "v":
                        vtrans_batch(p, 0, dst)
                        vtrans_batch(p, 4, dst)

            # ---- filler machinery (global slot counter, absolute gates) --
            fillers = []        # (min_slot, thunk)
            slot_counter = [0]

            def slot_cb():
                slot_counter[0] += 1
                s = slot_counter[0]
                for i in range(len(fillers)):
                    if fillers[i][0] <= s:
                        th = fillers[i][1]
                        del fillers[i]
                        th()
                        break

            def drain_fillers():
                while fillers:
                    fillers.pop(0)[1]()

            # t2=1 projections for pairs 0,1 -> early attn(0) fillers
            for gi, (inp, p) in enumerate(
                    (("q", 0), ("q", 1), ("k", 0), ("k", 1),
                     ("v", 0), ("v", 1))):
                dst = vts[p] if inp == "v" else (qT if inp == "q" else kT)[p]
                fillers.append(
                    (2 + gi, lambda inp=inp, p=p, dst=dst:
                     proj_group(inp, p, 1, dst, "a", True)))
            for gi, (p, i0) in enumerate(((0, 8), (0, 12), (1, 8), (1, 12))):
                fillers.append(
                    (9 + gi, lambda p=p, i0=i0: vtrans_batch(p, i0, vts[p])))

            # pair-2 projections (xb) as later attn(0) fillers
            def make_projB_fillers():
                vtile = vtp.tile([128, SEG], dt.bfloat16, tag="vT",
                                 name="vtB")
                gates = {"q": 14, "k": 22, "v": 41}
                for inp in ("q", "k", "v"):
                    dst = {"q": qT[2], "k": kT[2], "v": vtile}[inp]
                    g = gates[inp]
                    for t2 in range(2):
                        fillers.append(
                            (g + 3 * t2, lambda inp=inp, t2=t2, dst=dst:
                             proj_group(inp, 2, t2, dst, "b", False)))
                for bi, i0 in enumerate((0, 4, 8, 12)):
                    fillers.append(
                        (47 + 2 * bi, lambda i0=i0: vtrans_batch(2, i0,
                                                                 vtile)))

            # out-projection chunk fillers
            def y_unit(ydram, pairs, m, cast_on_act=False):
                def th():
                    accy = ps_b.tile([128, 2 * NQ], dt.float32, tag="b",
                                     name="accy")
                    for jc in range(2):
                        for idx, p in enumerate(pairs):
                            nc.tensor.matmul(
                                accy[:, NQ * jc:NQ * (jc + 1)],
                                oT[p][:, 128 * m:128 * (m + 1)],
                                woT[p][:, NQ * jc:NQ * (jc + 1)],
                                start=(idx == 0), stop=(idx == len(pairs) - 1))
                    t = ysb.tile([128, 2 * NQ], dt.bfloat16, tag="ysb")
                    if cast_on_act:
                        nc.scalar.copy(t, accy)
                    else:
                        nc.vector.tensor_copy(t, accy)
                    nc.sync.dma_start(
                        out=ydram[128 * m:128 * (m + 1), :], in_=t)
                return th

            # ---- normalization (DMA-scatter batched reciprocal) ----
            def drow(j, h):
                return 32 * (2 * (j % 2) + h), NQ * (j // 2)

            dens = [dnp.tile([128, 2 * NQ], dt.bfloat16, tag="dens",
                             name=f"dens{p}") for p in range(3)]
            dsp = [dsp_p.tile([32, 8 * 16], dt.bfloat16, tag="dsp",
                              name=f"dsp{p}") for p in range(3)]
            rsp = [dsp_p.tile([32, 8 * 16], dt.float32, tag="rsp",
                              name=f"rsp{p}") for p in range(3)]

            def recip_batch(p, us):
                # spread each den row over 32 partitions (64B DMA lines) so
                # ONE short-free-dim reciprocal covers the batch
                def th():
                    for u in us:
                        r, c = drow(u // 2, u % 2)
                        nc.gpsimd.dma_start(
                            out=dsp[p][:, 16 * u:16 * (u + 1)],
                            in_=dens[p][r:r + 1, c:c + NQ])
                    u0, u1 = us[0], us[-1] + 1
                    nc.vector.reciprocal(rsp[p][:, 16 * u0:16 * u1],
                                         dsp[p][:, 16 * u0:16 * u1])
                return th

            def norm_bc_unit(p, j, rb2s):
                def th():
                    rb2 = rbp.tile([128, NQ], dt.float32, tag="rb")
                    rb2s[j] = rb2
                    for h in range(2):
                        u = 2 * j + h
                        rc = rcq.tile([1, NQ], dt.float32, tag="rc")
                        nc.gpsimd.dma_start(
                            out=rc, in_=rsp[p][:, 16 * u:16 * (u + 1)])
                        if h == 0:
                            nc.gpsimd.partition_broadcast(rb2[0:64, :], rc)
                        else:
                            rbt = rbtp.tile([64, NQ], dt.float32,
                                            tag="rbt")
                            nc.gpsimd.partition_broadcast(rbt, rc)
                            nc.gpsimd.dma_start(out=rb2[64:128, :], in_=rbt)
                return th

            def norm_mul_unit(p, j, rb2s):
                def th():
                    sl = oT[p][:, NQ * j:NQ * (j + 1)]
                    nc.vector.tensor_mul(sl, sl, rb2s.pop(j))
                return th

            def norm_fillers(p, base, js):
                rb2s = {}
                ents = [(base, recip_batch(p, [u for j in js
                                               for u in (2 * j, 2 * j + 1)]))]
                for i, j in enumerate(js):
                    ents.append((base + 2 + 2 * i, norm_bc_unit(p, j, rb2s)))
                    ents.append((base + 5 + 2 * i, norm_mul_unit(p, j, rb2s)))
                ents.sort(key=lambda e: e[0])
                return ents

            # ---- attention: flat chunk stream with cross-boundary S
            # lookahead (keeps ACT fed across j/pair boundaries) ----
            def s_emit(p, j, ci):
                s = ps_s.tile([128, 2 * NQ], dt.float32, tag="s", name="s")
                c0 = max(ci - 4 * j, 0) * 128
                nc.tensor.matmul(
                    s[:, c0:NQ],
                    kT[p][0:64, 128 * ci:128 * (ci + 1)],
                    qT[p][0:64, NQ * j + c0:NQ * (j + 1)],
                    start=True, stop=True)
                nc.tensor.matmul(
                    s[:, NQ + c0:2 * NQ],
                    kT[p][64:128, 128 * ci:128 * (ci + 1)],
                    qT[p][64:128, NQ * j + c0:NQ * (j + 1)],
                    start=True, stop=True)
                di = ci - 4 * j
                if di >= 0:
                    sv = s.rearrange("p (h x) -> p h x", h=2)[
                        :, :, c0:c0 + 128]
                    nc.vector.tensor_add(
                        sv, sv,
                        maskadd.rearrange("p (h x) -> p h x", h=2))
                return s

            chunks = []         # flat stream: (p, j, ci, nchunks)
            hooks = {}          # (p, j) -> after_j hook
            for p in range(3):
                js = range(NTQ) if p < 2 else (2, 1, 0, 3)
                for j in js:
                    n = 4 * (j + 1)
                    for ci in range(n):
                        chunks.append((p, j, ci, n))

            def attn_all():
                accs = {}
                s_tiles = {}
                first = chunks[0]
                s_tiles[first[:3]] = s_emit(*first[:3])
                for idx, (p, j, ci, n) in enumerate(chunks):
                    if ci == 0:
                        acc0 = ps_o.tile([128, NQ], dt.float32, tag="acc",
                                         name="acc0")
                        acc1 = ps_o.tile([128, NQ], dt.float32, tag="acc",
                                         name="acc1")
                        accs[(p, j)] = (acc0, acc1)
                    acc0, acc1 = accs[(p, j)]
                    s = s_tiles.pop((p, j, ci))
                    pt = ptp.tile([128, 2 * NQ], dt.bfloat16, tag="pt",
                                  name="pt")
                    # cols [0, 128*di) of a diagonal chunk are fully masked:
                    # exp and the PV matmuls skip them entirely
                    c0 = max(ci - 4 * j, 0) * 128
                    if c0 == 0:
                        nc.scalar.activation(
                            pt, s, mybir.ActivationFunctionType.Exp,
                            bias=0.0, scale=0.125)
                    else:
                        pv = pt.rearrange("p (h x) -> p h x", h=2)[
                            :, :, c0:NQ]
                        sv = s.rearrange("p (h x) -> p h x", h=2)[
                            :, :, c0:NQ]
                        nc.scalar.activation(
                            pv, sv, mybir.ActivationFunctionType.Exp,
                            bias=0.0, scale=0.125)
                    if idx + 1 < len(chunks):
                        nxt = chunks[idx + 1]
                        s_tiles[nxt[:3]] = s_emit(*nxt[:3])
                    last = (ci == n - 1)
                    nc.tensor.matmul(
                        acc0[0:65, c0:NQ],
                        vn[p][:, 130 * ci:130 * ci + 65],
                        pt[:, c0:NQ], start=(ci == 0), stop=last)
                    nc.tensor.matmul(
                        acc1[0:65, c0:NQ],
                        vn[p][:, 130 * ci + 65:130 * ci + 130],
                        pt[:, NQ + c0:2 * NQ], start=(ci == 0), stop=last)
                    slot_cb()
                    if last:
                        for h, acc in enumerate((acc0, acc1)):
                            nc.vector.tensor_copy(
                                oT[p][64 * h:64 * h + 64,
                                      NQ * j:NQ * (j + 1)],
                                acc[0:64, :])
                            r, c = drow(j, h)
                            nc.vector.tensor_copy(
                                dens[p][r:r + 1, c:c + NQ], acc[64:65, :])
                        accs.pop((p, j))
                        hk = hooks.get((p, j))
                        if hk is not None:
                            hk()

            # norm/y filler schedule: pairs 0/1 normalize per j-half;
            # pair 2 runs j descending and normalizes each j right after
            # it completes, so y1 streams inside the pair (small tail)
            def q_norm(p, js, yd=None, pairs=None, ms=(), lag=1):
                def hk():
                    base = slot_counter[0] + lag
                    fillers.extend(norm_fillers(p, base, js))
                    yb = base + 4 + 2 * len(js)
                    for i, m in enumerate(ms):
                        fillers.append((yb + i, y_unit(yd, pairs, m)))
                return hk

            hooks[(0, 1)] = q_norm(0, [0, 1])
            hooks[(0, 3)] = q_norm(0, [2, 3], lag=5)
            hooks[(1, 1)] = q_norm(1, [0, 1], y0, (0, 1), range(8))
            hooks[(1, 3)] = q_norm(1, [2, 3], y0, (0, 1), range(8, 16),
                                   lag=5)
            hooks[(2, 2)] = q_norm(2, [2], y1, (2,), range(8, 12))
            hooks[(2, 1)] = q_norm(2, [1], y1, (2,), range(4, 8))
            hooks[(2, 0)] = q_norm(2, [0], y1, (2,), range(0, 4))
            hooks[(2, 3)] = q_norm(2, [3], y1, (2,), range(12, 16))

            make_projB_fillers()
            attn_all()
            drain_fillers()

    nc.compile()
    return nc


def _get_program():
    if "nc" not in _CACHE:
        _CACHE["nc"] = _build_program()
    return _CACHE["nc"]


def _sbuf_layout(xT):
    """[1024, 2048] (e, pos) -> [128, 2*8*1024]: pos-half major, ec, pos."""
    # [ec, 128, t2, 1024] -> [128, t2, ec, 1024]
    return np.ascontiguousarray(
        xT.reshape(ECH, 128, 2, 1024).transpose(1, 2, 0, 3).reshape(
            128, 2 * ECH * 1024))


def _prep_inputs(query, key, value, Wq, bq, Wk, bk, Wv, bv, Wo, bo):
    """Build the 8 per-core input maps (host-side slicing + bf16 cast)."""
    q = np.asarray(query, np.float32).reshape(8192, 1024).astype(BF16)
    k = np.asarray(key, np.float32).reshape(8192, 1024).astype(BF16)
    v = np.asarray(value, np.float32).reshape(8192, 1024).astype(BF16)
    wq = np.asarray(Wq, np.float32).astype(BF16)
    wk = np.asarray(Wk, np.float32).astype(BF16)
    wv = np.asarray(Wv, np.float32).astype(BF16)
    wo_f = np.asarray(Wo, np.float32).astype(BF16)
    bqf = np.asarray(bq, np.float32)
    bkf = np.asarray(bk, np.float32)
    bvf = np.asarray(bv, np.float32)

    qT, kT, vT = q.T, k.T, v.T  # [1024, 8192] views
    in_maps = []
    for c in range(8):
        a, sc, b, rc = c % 2, c // 2, c % 4, c // 4
        rows_g0 = slice(2048 * sc, 2048 * (sc + 1))
        rows_g1 = slice(4096 * rc + 1, 4096 * (rc + 1), 2)
        hrows = np.r_[256 * a:256 * a + 256, 512 + 128 * b:512 + 128 * b + 128]

        def wlay(w):
            # [1024, 384] -> [128, 8*384] (ec-blocked)
            return np.ascontiguousarray(
                w.reshape(ECH, 128, 384).transpose(1, 0, 2).reshape(128, -1))

        wov = np.ascontiguousarray(wo_f[:, hrows].T)  # [384, 1024]
        ballv = np.stack([np.ascontiguousarray(bf[hrows]).reshape(3, 128)
                          for bf in (bqf, bkf, bvf)], axis=0)  # [3,3,128]
        # ball[128, 9]: col 3i+p = input i, pair p
        ballv = np.ascontiguousarray(ballv.reshape(9, 128).T)
        m = {
            "xa_q": _sbuf_layout(np.ascontiguousarray(qT[:, rows_g0])),
            "xa_k": _sbuf_layout(np.ascontiguousarray(kT[:, rows_g0])),
            "xa_v": _sbuf_layout(np.ascontiguousarray(vT[:, rows_g0])),
            "xb_q": _sbuf_layout(np.ascontiguousarray(qT[:, rows_g1])),
            "xb_k": _sbuf_layout(np.ascontiguousarray(kT[:, rows_g1])),
            "xb_v": _sbuf_layout(np.ascontiguousarray(vT[:, rows_g1])),
            "wq": wlay(np.ascontiguousarray(wq[hrows].T)),
            "wk": wlay(np.ascontiguousarray(wk[hrows].T)),
            "wv": wlay(np.ascontiguousarray(wv[hrows].T)),
            "wo": np.ascontiguousarray(
                wov.reshape(3, 128, 1024).transpose(1, 0, 2).reshape(128, -1)),
            "ball": ballv,
        }
        in_maps.append(m)
    return in_maps


def _combine(results, bo):
    y = np.zeros((8192, 1024), np.float32)
    for c in range(8):
        sc, rc = c // 2, c // 4
        y[2048 * sc:2048 * (sc + 1)] += results[c]["y0"].astype(np.float32)
        y[4096 * rc + 1:4096 * (rc + 1):2] += results[c]["y1"].astype(np.float32)
    y += np.asarray(bo, np.float32)
    return y.reshape(1, 8192, 1024)


def kernel(query, key, value, Wq, bq, Wk, bk, Wv, bv, Wo, bo,
           _trace=False, _trace_cores=None):
    from concourse import bass_utils
    nc = _get_program()
    in_maps = _prep_inputs(query, key, value, Wq, bq, Wk, bk, Wv, bv, Wo, bo)
    res = bass_utils.run_bass_kernel_spmd(
        nc, in_maps, core_ids=list(range(8)),
        trace=_trace, trace_cores=_trace_cores)
    _CACHE["last_results"] = res
    return _combine(res.results, bo)
